# revision 62
# baseline (speedup 1.0000x reference)
"""Trainium2 Bass kernel for nn_MultiHeadAttention_4810363372776 (linear attention).

Sharding: data-parallel over batch (4) x tensor-parallel over head groups (2).
Core i handles batch i//2, heads [8*(i%2), 8*(i%2)+8). Each core computes its
partial output projection in bf16; the host sums the two head-group partials
per batch and adds the output bias.

Design notes:
- Activations arrive pre-transposed ([d, s] tiled) and pre-cast (q/k fp8,
  v bf16) from the host, so the kernel does zero PE transposes.
- q/k projections run fp8 DoubleRow (2x); v and the output projection stay
  bf16 (fp8 there puts ~4% error on the random-sign contractions, over the
  2e-2 budget).
- kv^T is accumulated directly in transposed form (lhsT=vf, rhs=kf) together
  with ksum^T; phase 2 then folds Wo through kv once (M = kv_blockdiag @ WoT)
  so the output projection consumes exp(q_hat) directly — no per-tile
  numerator matmul and no ctx transpose.
- Per-head normalization: denominators accumulate into an [8, s] PSUM tile
  (zero-padded ksum selectors), reciprocal on a 128-free-dim layout, then a
  K=8 selector matmul broadcasts 1/denom to each head's 64 o-rows.
- A PE warm-up burst bridges DMA-ring init so the HAM clock gate stays at
  2.4 GHz when real matmuls start; DMA queues are assigned/ordered so the
  first k-tile's inputs arrive as early as possible.
"""

import functools
import numpy as np

B, S, D, H = 4, 4096, 1024, 16
DK = D // H          # 64
OG = D // 2          # 512 per-core head-group width (8 heads)
NCORES = 8
SCALE = 1.0 / 8.0    # 1/sqrt(DK)
NT = S // 128        # 32 s-tiles
SM = 512             # s-chunk per input DMA
NCH = S // SM        # 8 chunks
NU = SM // 128       # tiles per chunk


@functools.lru_cache(maxsize=2)
def _build(kv_bias=False):
    import concourse.bass as bass  # noqa: F401
    from concourse import bacc
    import concourse.mybir as mybir
    import concourse.tile as tile
    from contextlib import ExitStack

    f32 = mybir.dt.float32
    bf16 = mybir.dt.bfloat16
    fp8 = mybir.dt.float8e4
    DR = mybir.MatmulPerfMode.DoubleRow
    EXP = mybir.ActivationFunctionType.Exp
    COPY = mybir.ActivationFunctionType.Copy
    AXX = mybir.AxisListType.X
    ADD = mybir.AluOpType.add

    nc = bacc.Bacc()

    # xq[p, a, t, s'] = q[SM*a + s', 128*t + p] — per-partition-contiguous chunks
    xq = nc.declare_dram_parameter("xq", [128, NCH, 8, SM], fp8, isOutput=False)
    # xk/xv are s-tile-major: x[p, st, t, s'] = x[128*st + s', 128*t + p], so
    # chunk 0 can stream in per-tile pieces (fast first matmul)
    xk = nc.declare_dram_parameter("xk", [128, NT, 8, 128], fp8, isOutput=False)
    xv = nc.declare_dram_parameter("xv", [128, NT, 8, 128], bf16, isOutput=False)
    # w*[p, t, o] = W[og_slice, :].T[128*t + p, o]
    wq = nc.declare_dram_parameter("wq", [128, 8, OG], fp8, isOutput=False)
    wk = nc.declare_dram_parameter("wk", [128, 8, OG], fp8, isOutput=False)
    wv = nc.declare_dram_parameter("wv", [128, 8, OG], bf16, isOutput=False)
    # wo[p, t, d] = Wo[:, og_slice].T[128*t + p, d]
    wo = nc.declare_dram_parameter("wo", [128, 4, D], bf16, isOutput=False)
    bqsp = nc.declare_dram_parameter("bqs", [128, 4], f32, isOutput=False)
    bkp = nc.declare_dram_parameter("bk", [1, OG], f32, isOutput=False)
    bvp = nc.declare_dram_parameter("bv", [1, OG], f32, isOutput=False)
    maskp = nc.declare_dram_parameter("maskf", [128, NT], f32, isOutput=False)
    sel8p = nc.declare_dram_parameter("sel8", [8, 4, 128], bf16, isOutput=False)
    # block-diag 0/1 mask [o', e'] and per-head-slot 0/1 mask [o', p, j]
    bdmp = nc.declare_dram_parameter("bdmask", [128, 128], bf16, isOutput=False)
    m8p = nc.declare_dram_parameter("mask8", [128, 4, 8], bf16, isOutput=False)
    out = nc.declare_dram_parameter("out", [S, D], bf16, isOutput=True)

    with tile.TileContext(nc) as tc:
        with ExitStack() as ctx:
            singles = ctx.enter_context(tc.tile_pool(name="singles", bufs=1))

            # weight loads in strict need-order: wk (in d-block pieces so the
            # first matmul chain can start on piece 0) then the first xk chunk
            # on the scalar queue; wv/wq are emitted after chunk 0's loads,
            # wo only at the interphase
            wk_sb = singles.tile([128, 8, OG], fp8, tag="wk")
            nc.scalar.dma_start(out=wk_sb[:, 0:4, :], in_=wk[:, 0:4, :])
            nc.gpsimd.dma_start(out=wk_sb[:, 4:8, :], in_=wk[:, 4:8, :])
            wv_sb = singles.tile([128, 8, OG], bf16, tag="wv")
            wq_sb = singles.tile([128, 8, OG], fp8, tag="wq")
            wo_sb = singles.tile([128, 4, D], bf16, tag="wo")

            bqs_sb = singles.tile([128, 4], f32, tag="bqs")
            nc.sync.dma_start(out=bqs_sb, in_=bqsp[:, :])
            if kv_bias:
                bk_bc = singles.tile([128, OG], f32, tag="bk_bc")
                nc.gpsimd.dma_start(out=bk_bc, in_=bkp[:, :].partition_broadcast(128))
                bv_bc = singles.tile([128, OG], f32, tag="bv_bc")
                nc.gpsimd.dma_start(out=bv_bc, in_=bvp[:, :].partition_broadcast(128))
            mask_sb = singles.tile([128, NT], f32, tag="mask")
            nc.sync.dma_start(out=mask_sb, in_=maskp[:, :])

            # constants
            ones_s = singles.tile([128, 1], bf16, tag="ones_s")
            nc.vector.memset(ones_s, 1.0)
            # sel8[:, p, :]: partition j broadcasts 1/denom of head j to the
            # o-rows of pair p ([0:64] -> head 2p, [64:128] -> head 2p+1)
            sel8 = singles.tile([8, 4, 128], bf16, tag="sel8")
            nc.sync.dma_start(out=sel8, in_=sel8p[:, :, :])
            bdmask = singles.tile([128, 128], bf16, tag="bdmask")
            nc.sync.dma_start(out=bdmask, in_=bdmp[:, :])
            mask8 = singles.tile([128, 4, 8], bf16, tag="mask8")
            nc.sync.dma_start(out=mask8, in_=m8p[:, :, :])

            # exp(q_hat * scale), stored [o (4 blocks of 128 = head pairs), s]
            ET = singles.tile([128, 4, S], bf16, tag="ET")
            # fused (block-diag kv) @ WoT, plus per-pair head-slot ksum
            M_sb = singles.tile([128, 4, D], bf16, tag="M")
            # ksum_bd8[o', p, j] = ksum[o'] if head(p, o') == j else 0
            ksum_bd8 = singles.tile([128, 4, 8], bf16, tag="ksum_bd8")

            # ---------------- phase 1 ----------------
            with ExitStack() as p1:
                pacc_pool = p1.enter_context(tc.tile_pool(name="pacc", bufs=1, space="PSUM"))
                # kv^T accumulated per head pair: [e', pair, o'] (with cross-head
                # garbage at off-diagonal 64-blocks, masked out later)
                kvT_ps = pacc_pool.tile([128, 4, 128], f32, tag="kvT", name="kvT")
                # ksum^T accumulated per pair: [1, pair, o']
                ksT_ps = pacc_pool.tile([1, 4, 128], f32, tag="ksT", name="ksT")

                # warm-up burst: keep the PE busy (and the HAM clock
                # un-throttled) while the DMA rings initialize and the first
                # inputs stream in; results land in kvT_ps, whose real
                # accumulation chain clears the bank on its first matmul
                warm_a = singles.tile([128, 128], bf16, tag="warm_a")
                nc.vector.memset(warm_a, 0.0)
                warm_b = singles.tile([128, 512], bf16, tag="warm_b")
                nc.vector.memset(warm_b, 0.0)
                for _ in range(34):
                    nc.tensor.matmul(kvT_ps, warm_a, warm_b, start=True, stop=True,
                                     skip_group_check=True)

                with ExitStack() as p1a:
                    xin_pool = p1a.enter_context(tc.tile_pool(name="xin", bufs=4))
                    kvf_pool = p1a.enter_context(tc.tile_pool(name="kvf", bufs=3))
                    pkv_pool = p1a.enter_context(tc.tile_pool(name="pkv", bufs=4, space="PSUM"))
                    pq_pool = p1a.enter_context(tc.tile_pool(name="pq", bufs=2, space="PSUM"))

                    pending = None  # (kf, vf, st) deferred kv accumulation

                    def flush_kvT_one(pending, p):
                        kf, vf, pst = pending
                        nc.tensor.matmul(
                            kvT_ps[:, p, :],
                            vf[:, p, :],
                            kf[:, 2 * p:2 * p + 2, :],
                            start=(pst == 0 and p == 0),
                            stop=(pst == NT - 1),
                            skip_group_check=True,
                        )

                    def flush_ksT(pending):
                        kf, vf, pst = pending
                        nc.tensor.matmul(
                            ksT_ps,
                            ones_s,
                            kf,
                            start=(pst == 0),
                            stop=(pst == NT - 1),
                            skip_group_check=True,
                        )

                    def flush_kv(pending):
                        for p in range(4):
                            flush_kvT_one(pending, p)
                        flush_ksT(pending)

                    for a in range(NCH):
                        # xk/xv are s-tile-major; chunk 0 streams in per-tile
                        # pieces on the critical path (scalar queue, right
                        # behind wk) so the first matmul starts ASAP
                        xk_sb = xin_pool.tile([128, NU, 8, 128], fp8, tag="xk")
                        xv_sb = xin_pool.tile([128, NU, 8, 128], bf16, tag="xv")
                        if a == 0:
                            for u in range(NU):
                                nc.scalar.dma_start(out=xk_sb[:, u], in_=xk[:, u])
                                nc.gpsimd.dma_start(out=xv_sb[:, u], in_=xv[:, u])
                        else:
                            nc.sync.dma_start(out=xk_sb, in_=xk[:, a * NU:(a + 1) * NU])
                            nc.gpsimd.dma_start(out=xv_sb, in_=xv[:, a * NU:(a + 1) * NU])
                        # chunk 0's q input and wq ride the sync queue (their
                        # first consumer is the q-projection ~16us in); later
                        # xq chunks use gpsimd
                        xq_sb = xin_pool.tile([128, 8, SM], fp8, tag="xq")
                        (nc.sync if a == 0 else nc.gpsimd).dma_start(out=xq_sb, in_=xq[:, a, :, :])
                        if a == 0:
                            # remaining weights, off the critical path
                            nc.scalar.dma_start(out=wv_sb, in_=wv[:, :, :])
                            nc.sync.dma_start(out=wq_sb, in_=wq[:, :, :])
                        elif a == 1:
                            nc.sync.dma_start(out=wo_sb, in_=wo[:, :, :])

                        for u in range(NU):
                            st = a * NU + u

                            # k projection -> [s, og]
                            pk = pkv_pool.tile([128, OG], f32, tag="pkv")
                            for t2 in range(4):
                                nc.tensor.matmul(pk, xk_sb[:, u, 2 * t2:2 * t2 + 2, :],
                                                 wk_sb[:, 2 * t2:2 * t2 + 2, :],
                                                 start=(t2 == 0), stop=(t2 == 3), perf_mode=DR)
                            if kv_bias:
                                nc.vector.tensor_add(pk, pk, bk_bc)
                            ek = kvf_pool.tile([128, OG], bf16, tag="ek")
                            nc.scalar.activation(ek, pk, EXP, scale=SCALE)
                            rows = kvf_pool.tile([128, 8], f32, tag="rows")
                            nc.vector.tensor_reduce(rows, ek.rearrange("p (h e) -> p h e", h=8), axis=AXX, op=ADD)
                            nc.vector.reciprocal(rows, rows)
                            nc.vector.tensor_scalar_mul(rows, rows, mask_sb[:, st:st + 1])
                            kf = kvf_pool.tile([128, 8, DK], bf16, tag="kf")
                            nc.vector.tensor_mul(
                                kf,
                                ek.rearrange("p (h e) -> p h e", h=8),
                                rows[:, :, None].to_broadcast([128, 8, DK]),
                            )

                            # v projection -> [s, og]
                            pv = pkv_pool.tile([128, OG], f32, tag="pkv")
                            for t in range(8):
                                nc.tensor.matmul(pv, xv_sb[:, u, t, :], wv_sb[:, t, :],
                                                 start=(t == 0), stop=(t == 7))
                            if kv_bias:
                                nc.vector.tensor_add(pv, pv, bv_bc)
                            vf = kvf_pool.tile([128, 4, 128], bf16, tag="vf")
                            nc.scalar.activation(vf, pv.rearrange("p (j e) -> p j e", j=4),
                                                 COPY, scale=mask_sb[:, st:st + 1])

                            # deferred kv/ksum accumulation for the previous s-tile
                            if pending is not None:
                                flush_kv(pending)
                            pending = (kf, vf, st)

                        # q projection for the chunk, output transposed [o, s]
                        for ob in range(4):
                            pq = pq_pool.tile([128, SM], f32, tag="pq")
                            for t2 in range(4):
                                nc.tensor.matmul(pq, wq_sb[:, 2 * t2:2 * t2 + 2, ob * 128:(ob + 1) * 128],
                                                 xq_sb[:, 2 * t2:2 * t2 + 2, :],
                                                 start=(t2 == 0), stop=(t2 == 3), perf_mode=DR)
                            nc.scalar.activation(ET[:, ob, a * SM:(a + 1) * SM], pq, EXP,
                                                 bias=bqs_sb[:, ob:ob + 1], scale=SCALE)

                    flush_kv(pending)

                # ---------------- interphase ----------------
                pks_pool = p1.enter_context(tc.tile_pool(name="pks", bufs=1, space="PSUM"))
                pm_pool = p1.enter_context(tc.tile_pool(name="pm", bufs=4, space="PSUM"))

                # block-diagonal kv^T: one masked multiply zeroes the
                # cross-head 64-blocks
                kvbd = singles.tile([128, 4, 128], bf16, tag="kvbd")
                nc.vector.tensor_mul(kvbd, kvT_ps,
                                     bdmask[:, None, :].to_broadcast([128, 4, 128]))

                # transpose ksum^T [1, o'] -> [o', 1] via K=1 matmul, then
                # scatter into per-head columns of ksum_bd8 via a masked mul
                ksT_sb = singles.tile([1, 4, 128], bf16, tag="ksT_sb")
                nc.scalar.copy(out=ksT_sb, in_=ksT_ps)
                pks = pks_pool.tile([128, 4, 1], f32, tag="pks")
                for p in range(4):
                    nc.tensor.matmul(pks[:, p, :], ksT_sb[0:1, p, :], ones_s[0:1, :], start=True, stop=True)
                nc.vector.tensor_mul(ksum_bd8,
                                     pks.to_broadcast([128, 4, 8]),
                                     mask8)

                # M = kv_bd @ WoT (per pair block); evacs alternate engines so
                # the matmul chain isn't paced by a single engine's copies
                for p in range(4):
                    for half in range(2):
                        pm = pm_pool.tile([128, 512], f32, tag="pm")
                        nc.tensor.matmul(pm, kvbd[:, p, :], wo_sb[:, p, half * 512:(half + 1) * 512],
                                         start=True, stop=True)
                        dst = M_sb[:, p, half * 512:(half + 1) * 512]
                        if half == 0:
                            nc.scalar.copy(out=dst, in_=pm)
                        else:
                            nc.vector.tensor_copy(dst, pm)

            # ---------------- phase 2 ----------------
            with ExitStack() as p2s:
                sb2 = p2s.enter_context(tc.tile_pool(name="sb2", bufs=4))
                pd_pool = p2s.enter_context(tc.tile_pool(name="pd", bufs=2, space="PSUM"))
                prd_pool = p2s.enter_context(tc.tile_pool(name="prd", bufs=2, space="PSUM"))
                pout_pool = p2s.enter_context(tc.tile_pool(name="pout", bufs=2, space="PSUM"))

                for st in range(NT):
                    s0 = st * 128
                    # per-head denominators accumulated into one [8, s'] tile:
                    # pd8[j, s'] = <ksum_(head j), ET_(head j)[:, s']>
                    pd8 = pd_pool.tile([8, 128], f32, tag="pd", name="pd")
                    for p in range(4):
                        nc.tensor.matmul(pd8, ksum_bd8[:, p, :], ET[:, p, s0:s0 + 128],
                                         start=(p == 0), stop=(p == 3))
                    rs_pre = sb2.tile([8, 128], f32, tag="rspre", name="rspre")
                    nc.scalar.activation(rs_pre, pd8, COPY, bias=1e-6)
                    rs8 = sb2.tile([8, 128], bf16, tag="rs", name="rs")
                    with nc.allow_low_precision(reason="1/denom in bf16 is within tolerance"):
                        nc.vector.reciprocal(rs8, rs_pre)
                    # broadcast 1/denom to the 64 o-rows of each head
                    prd = prd_pool.tile([128, 4, 128], f32, tag="prd", name="prd")
                    for p in range(4):
                        nc.tensor.matmul(prd[:, p, :], sel8[:, p, :], rs8, start=True, stop=True)
                    # normalized exp(q) features
                    ets = sb2.tile([128, 4, 128], bf16, tag="ets", name="ets")
                    nc.vector.tensor_mul(ets, ET[:, :, s0:s0 + 128], prd)
                    # fused output projection
                    pout = pout_pool.tile([128, 2, 512], f32, tag="pout", name="pout")
                    for half in range(2):
                        for p in range(4):
                            nc.tensor.matmul(pout[:, half, :], ets[:, p, :],
                                             M_sb[:, p, half * 512:(half + 1) * 512],
                                             start=(p == 0), stop=(p == 3))
                    outsb = sb2.tile([128, D], bf16, tag="outsb", name="outsb")
                    nc.scalar.copy(out=outsb.rearrange("p (j e) -> p j e", j=2), in_=pout)
                    nc.sync.dma_start(out=out[s0:s0 + 128, :], in_=outsb)

    nc.compile()
    return nc


_LAST_RESULT = None


def _ensure_ntff_hook():
    """Make `antenv.axon_hooks` importable so BASS_TRACE profiling works.

    Some images ship a minimal `antenv` stub without `axon_hooks`; the boot
    shim then degrades silently and bass_utils crashes on import when
    trace=True under axon. Inject the module and install the ctypes NTFF
    hook if possible. No-op when the real module exists.
    """
    try:
        from antenv import axon_hooks  # noqa: F401
        return
    except ImportError:
        pass
    import sys
    import types
    try:
        import antenv
    except ImportError:
        return
    mod = types.ModuleType("antenv.axon_hooks")
    mod._hook = None

    def set_axon_ntff_profile_hook(hook):
        mod._hook = hook

    def get_axon_ntff_profile_hook():
        return mod._hook

    mod.set_axon_ntff_profile_hook = set_axon_ntff_profile_hook
    mod.get_axon_ntff_profile_hook = get_axon_ntff_profile_hook
    sys.modules["antenv.axon_hooks"] = mod
    antenv.axon_hooks = mod
    try:
        from trn_agent_boot.trn_boot import _ntff_profile_via_ctypes

        hook = _ntff_profile_via_ctypes("/opt/axon/libaxon_pjrt.so")
        if hook is not None:
            set_axon_ntff_profile_hook(hook)
    except Exception:
        pass


def kernel(q, k, v, mask, Wq, bq, Wk, bk, Wv, bv, Wo, bo):
    global _LAST_RESULT
    import ml_dtypes
    from concourse.bass_utils import run_bass_kernel_spmd

    _ensure_ntff_hook()

    q = np.asarray(q, np.float32)
    k = np.asarray(k, np.float32)
    v = np.asarray(v, np.float32)
    mask = np.asarray(mask)
    Wq = np.asarray(Wq, np.float32)
    Wk = np.asarray(Wk, np.float32)
    Wv = np.asarray(Wv, np.float32)
    Wo = np.asarray(Wo, np.float32)
    bq = np.asarray(bq, np.float32)
    bk = np.asarray(bk, np.float32)
    bv = np.asarray(bv, np.float32)
    bo = np.asarray(bo, np.float32)

    nc = _build(bool(np.any(bk) or np.any(bv)))

    bf = ml_dtypes.bfloat16
    f8 = ml_dtypes.float8_e4m3

    def xtile(x, dt, chunk=SM):
        # [S, D] -> [128, S//chunk, 8, chunk]: A[p, a, t, s'] = x[chunk*a + s', 128*t + p]
        xt = np.ascontiguousarray(x.T)
        return xt.reshape(8, 128, S // chunk, chunk).transpose(1, 2, 0, 3).astype(dt)

    def wtile(W, sl, nt, dt):
        # [128, nt, ncols]: w[p, t, o] = W[sl, :].T[128*t + p, o]
        wt = np.ascontiguousarray(W[sl, :].T) if sl is not None else W
        return wt.reshape(nt, 128, -1).transpose(1, 0, 2).astype(dt)

    sel8_host = np.zeros((8, 4, 128), bf)
    for p in range(4):
        sel8_host[2 * p, p, 0:64] = 1
        sel8_host[2 * p + 1, p, 64:128] = 1
    bdmask_host = np.zeros((128, 128), bf)
    bdmask_host[0:64, 0:64] = 1
    bdmask_host[64:128, 64:128] = 1
    mask8_host = np.zeros((128, 4, 8), bf)
    for p in range(4):
        mask8_host[0:64, p, 2 * p] = 1
        mask8_host[64:128, p, 2 * p + 1] = 1

    in_maps = []
    xcache = {}
    for core in range(NCORES):
        b, g = core // 2, core % 2
        sl = slice(g * OG, (g + 1) * OG)
        if b not in xcache:
            xcache[b] = (xtile(q[b], f8), xtile(k[b], f8, 128), xtile(v[b], bf, 128))
        xqh, xkh, xvh = xcache[b]
        maskf = mask[b, 0, 0, :].astype(np.float32).reshape(NT, 128).T.copy()
        in_maps.append({
            "xq": xqh,
            "xk": xkh,
            "xv": xvh,
            "wq": wtile(Wq, sl, 8, f8),
            "wk": wtile(Wk, sl, 8, f8),
            "wv": wtile(Wv, sl, 8, bf),
            "wo": wtile(np.ascontiguousarray(Wo[:, sl].T), None, 4, bf),
            "bqs": np.ascontiguousarray((bq[sl] * SCALE).reshape(4, 128).T),
            "bk": bk[sl].reshape(1, OG).copy(),
            "bv": bv[sl].reshape(1, OG).copy(),
            "maskf": maskf,
            "sel8": sel8_host,
            "bdmask": bdmask_host,
            "mask8": mask8_host,
        })

    res = run_bass_kernel_spmd(nc, in_maps, list(range(NCORES)))
    _LAST_RESULT = res

    outp = np.empty((B, S, D), np.float32)
    for b in range(B):
        outp[b] = (res.results[2 * b]["out"].astype(np.float32)
                   + res.results[2 * b + 1]["out"].astype(np.float32)
                   + bo[None, :])
    return outp
